# revision 6
# baseline (speedup 1.0000x reference)
"""ColorConstancy (multi-scale retinex) Trainium2 kernel.

Full-input contract: kernel(**inputs) takes the unsharded inputs from
setup_inputs() and returns the full (16, 3, 512, 512) float32 output.

Strategy (pure data parallel, batch sharded across 8 cores; 6 planes/core):
  log_img = ln(x + 1e-8)
  illum   = sum_s w_s * gauss2d_s(log_img)        (sigmas 2, 4, 8)
  refl    = log_img - illum
  out     = clip(exp((refl - mean) / (std_ddof1 + 1e-8)), 0, 1)

The 2-D Gaussian is separable: gauss2d_s(X) = U_s @ X @ U_s with U_s the
banded symmetric Toeplitz matrix of the 1-D kernel. Each pass is computed on
the TensorEngine as  pass(D) = D^T @ V  (lhsT = D blocks, rhs = V blocks), so
two passes give V^T X V = V X V with no explicit transposes. Folding
sqrt(w_s) into V_s makes illum a single PSUM accumulation in pass 2. The
banded structure limits each matmul's moving free dim to 128+2c columns.
Matmul operands are fp16 (full-rate PE, ~5e-4 rel precision); everything
else stays fp32.
"""

import numpy as np

N_CORES = 8
NPLANES = 6          # 2 batch images x 3 channels per core
H = W = 512
P = 128
NB = H // P          # 4 row blocks
CS = (6, 12, 24)     # band half-widths for sigma 2, 4, 8 (K = 13, 25, 49)
EPS = 1e-8
NPIX = H * W

_PROGRAM_CACHE = {}


def _ncol(kb, c):
    """Output column range that input row block kb touches through a band-c kernel."""
    return max(0, P * kb - c), min(W, P * (kb + 1) + c)


def build_program(reps=1):
    """Build + compile the per-core Bass program. reps>1 wraps the whole
    computation in a hardware loop (for timing by subtraction)."""
    import concourse.bacc as bacc
    import concourse.tile as tile
    from concourse import mybir, bass_isa

    f32 = mybir.dt.float32
    f16 = mybir.dt.float16
    AF = mybir.ActivationFunctionType

    # The activation-table chooser picks the first set containing each
    # function, which puts Ln in "natural_log" and Exp in "exp_and_others" and
    # reloads tables (~2.7us each) every plane. Narrow the cached table map so
    # only the combined "natural_log_exp_and_others" set provides Ln/Exp; then
    # one load serves the whole kernel. (The combined set genuinely contains
    # both functions; this only steers a valid choice.)
    from concourse.hw_specs import get_activation_tables
    _tabs = get_activation_tables("gen3")
    for _name, _fset in _tabs.items():
        if _name != "natural_log_exp_and_others":
            _fset.discard(AF.Ln)
            _fset.discard(AF.Exp)

    nc = bacc.Bacc("TRN2", target_bir_lowering=False, debug=False,
                   num_devices=N_CORES)
    x = nc.declare_dram_parameter("x", [NPLANES, H, W], f32, isOutput=False)
    vs = [nc.declare_dram_parameter(f"v{s}", [H, W], f16, isOutput=False)
          for s in range(3)]
    y = nc.declare_dram_parameter("y", [NPLANES, H, W], f32, isOutput=True)

    with tile.TileContext(nc) as tc:
        with (
            tc.tile_pool(name="consts", bufs=1) as consts,
            tc.tile_pool(name="xin", bufs=2) as xpool,
            tc.tile_pool(name="logp", bufs=2) as lpool,
            tc.tile_pool(name="log16", bufs=2) as l16pool,
            tc.tile_pool(name="apool", bufs=2) as apool,
            tc.tile_pool(name="refl", bufs=2) as rpool,
            tc.tile_pool(name="yout", bufs=2) as ypool,
            tc.tile_pool(name="small", bufs=2) as spool,
            tc.tile_pool(name="psA", bufs=6, space="PSUM") as psA,
            tc.tile_pool(name="psI", bufs=2, space="PSUM") as psIp,
        ):
            # Banded blur matrices, resident for the whole kernel.
            # Layout [p, kb, n]: matrix row = kb*128 + p.
            V16 = []
            for s in range(3):
                vt = consts.tile([P, NB, W], f16, tag=f"v{s}")
                nc.sync.dma_start(
                    out=vt, in_=vs[s].rearrange("(kb p) n -> p kb n", p=P))
                V16.append(vt)
            epst = consts.tile([P, 1], f32, tag="eps")
            nc.vector.memset(epst, EPS)

            def emit_planes():
                state = {}

                def front(p):
                    # load -> ln -> fp16 copy -> pass 1 (A_s = L^T V_s)
                    xt = xpool.tile([P, NB, W], f32, tag="x")
                    nc.sync.dma_start(
                        out=xt, in_=x[p].rearrange("(kb q) w -> q kb w", q=P))
                    lt = lpool.tile([P, NB, W], f32, tag="l")
                    nc.scalar.activation(out=lt, in_=xt, func=AF.Ln,
                                         bias=epst, scale=1.0)
                    l16 = l16pool.tile([P, NB, W], f16, tag="l16")
                    nc.gpsimd.tensor_copy(out=l16, in_=lt)

                    A16 = [apool.tile([P, NB, W], f16, tag=f"a{s}", name=f"a16_{s}")
                           for s in range(3)]
                    nevac = 0
                    for mb in range(NB):
                        ps = [psA.tile([P, W], f32, tag="ps", name=f"ps{i}")
                              for i in range(3)]
                        for s in range(3):
                            for kb in range(NB):
                                lo, hi = _ncol(kb, CS[s])
                                nc.tensor.matmul(
                                    ps[s][:, lo:hi],
                                    l16[:, kb, P * mb:P * (mb + 1)],
                                    V16[s][:, kb, lo:hi],
                                    start=(kb == 0), stop=(kb == NB - 1),
                                )
                        for s in range(3):
                            # evacuate PSUM -> SBUF fp16; 6 on DVE, 6 on ACT
                            if nevac % 2 == 1:
                                nc.scalar.copy(out=A16[s][:, mb, :], in_=ps[s])
                            else:
                                nc.vector.tensor_copy(out=A16[s][:, mb, :],
                                                      in_=ps[s])
                            nevac += 1
                    state[p] = (lt, A16)

                def back(p):
                    # pass 2 (illum = sum_s A_s^T V_s) -> refl -> stats -> out
                    lt, A16 = state.pop(p)
                    rt = rpool.tile([P, NB, W], f32, tag="r")
                    st6 = spool.tile([P, NB, 6], f32, tag="st6")
                    for mb in range(NB):
                        psi = psIp.tile([P, W], f32, tag="psi")
                        first = True
                        for s in range(3):
                            for kb in range(NB):
                                lo, hi = _ncol(kb, CS[s])
                                nc.tensor.matmul(
                                    psi[:, lo:hi],
                                    A16[s][:, kb, P * mb:P * (mb + 1)],
                                    V16[s][:, kb, lo:hi],
                                    start=first,
                                    stop=(s == 2 and kb == NB - 1),
                                )
                                first = False
                        nc.vector.tensor_sub(out=rt[:, mb, :],
                                             in0=lt[:, mb, :], in1=psi)
                        nc.vector.bn_stats(out=st6[:, mb, :], in_=rt[:, mb, :])

                    # plane-wide mean/var: per-partition bn stats, then an
                    # all-partition reduce of [mean_p, E[x^2]_p] on GpSimd.
                    mv = spool.tile([P, 2], f32, tag="mv")
                    nc.vector.bn_aggr(out=mv, in_=st6)
                    t2 = spool.tile([P, 2], f32, tag="t2")
                    nc.vector.tensor_mul(out=t2[:, 1:2], in0=mv[:, 0:1],
                                         in1=mv[:, 0:1])
                    nc.vector.tensor_add(out=t2[:, 1:2], in0=t2[:, 1:2],
                                         in1=mv[:, 1:2])
                    nc.vector.tensor_copy(out=t2[:, 0:1], in_=mv[:, 0:1])
                    red = spool.tile([P, 2], f32, tag="red")
                    nc.gpsimd.partition_all_reduce(
                        red, t2, channels=P, reduce_op=bass_isa.ReduceOp.add)

                    fin = spool.tile([P, 4], f32, tag="fin")
                    mean = fin[:, 0:1]
                    tmp = fin[:, 1:2]   # E[x^2] -> var -> std -> std+eps
                    rs = fin[:, 2:3]
                    nbv = fin[:, 3:4]
                    nc.vector.tensor_scalar_mul(out=mean, in0=red[:, 0:1],
                                                scalar1=1.0 / P)
                    nc.vector.tensor_scalar_mul(out=tmp, in0=red[:, 1:2],
                                                scalar1=1.0 / P)
                    sq = spool.tile([P, 1], f32, tag="sq")
                    nc.vector.tensor_mul(out=sq, in0=mean, in1=mean)
                    nc.vector.tensor_sub(out=tmp, in0=tmp, in1=sq)
                    # std = exp(0.5*ln(var * N/(N-1)))  (ddof=1), avoiding the
                    # sqrt table set; Ln/Exp share one ACT table set.
                    nc.scalar.activation(out=tmp, in_=tmp, func=AF.Ln,
                                         scale=float(NPIX) / (NPIX - 1))
                    nc.scalar.activation(out=tmp, in_=tmp, func=AF.Exp,
                                         scale=0.5)
                    nc.vector.tensor_scalar_add(out=tmp, in0=tmp, scalar1=EPS)
                    nc.vector.reciprocal(out=rs, in_=tmp)
                    nc.vector.tensor_mul(out=nbv, in0=mean, in1=rs)
                    nc.vector.tensor_scalar_mul(out=nbv, in0=nbv, scalar1=-1.0)

                    yt = ypool.tile([P, NB, W], f32, tag="y")
                    nc.scalar.activation(out=yt, in_=rt, func=AF.Exp,
                                         bias=nbv, scale=rs)
                    nc.gpsimd.tensor_scalar_min(out=yt, in0=yt, scalar1=1.0)
                    nc.sync.dma_start(
                        out=y[p].rearrange("(kb q) w -> q kb w", q=P), in_=yt)

                # software-pipelined: pass 1 of plane p overlaps pass 2 of p-1
                for p in range(NPLANES + 1):
                    if p < NPLANES:
                        front(p)
                    if p >= 1:
                        back(p - 1)

            if reps == 1:
                emit_planes()
            else:
                from concourse import mybir as _mb
                with tc.For_i(0, reps, 1,
                              hint_engines=(_mb.EngineType.PE,)):
                    emit_planes()

    nc.compile()
    return nc


def get_program(reps=1):
    if reps not in _PROGRAM_CACHE:
        _PROGRAM_CACHE[reps] = build_program(reps)
    return _PROGRAM_CACHE[reps]


def build_v_matrices(k0, k1, k2):
    """fp16 banded Toeplitz matrices sqrt(w_s) * toeplitz(u_s) from the
    reference's 2-D depthwise kernels (u_s = column sums of the normalized
    2-D kernel, exact by separability)."""
    w = np.array([1.0, 0.75, 0.5], dtype=np.float64)
    w /= w.sum()
    out = []
    for s, k2d in enumerate((k0, k1, k2)):
        g = np.asarray(k2d)[0, 0].astype(np.float64)
        u = g.sum(axis=0)
        c = len(u) // 2
        V = np.zeros((H, W), dtype=np.float64)
        for d in range(-c, c + 1):
            V += np.diag(np.full(H - abs(d), u[c + d]), k=d)
        V *= np.sqrt(w[s])
        out.append(V.astype(np.float16))
    return out


def kernel(rgb_image, k0, k1, k2):
    from concourse.bass_utils import run_bass_kernel_spmd

    nc = get_program()
    v16 = build_v_matrices(k0, k1, k2)
    xs = np.ascontiguousarray(np.asarray(rgb_image, dtype=np.float32))
    B = xs.shape[0]
    per_core = B // N_CORES
    in_maps = []
    for c in range(N_CORES):
        m = {"x": xs[c * per_core:(c + 1) * per_core].reshape(NPLANES, H, W)}
        for s in range(3):
            m[f"v{s}"] = v16[s]
        in_maps.append(m)
    res = run_bass_kernel_spmd(nc, in_maps, list(range(N_CORES)))
    out = np.empty((B, 3, H, W), dtype=np.float32)
    for c in range(N_CORES):
        out[c * per_core:(c + 1) * per_core] = (
            res.results[c]["y"].reshape(per_core, 3, H, W))
    return out


# revision 16
# speedup vs baseline: 3.3337x; 3.3337x over previous
"""ColorConstancy (multi-scale retinex) Trainium2 kernel.

Full-input contract: kernel(**inputs) takes the unsharded inputs from
setup_inputs() and returns the full (16, 3, 512, 512) float32 output.

Strategy (pure data parallel, batch sharded across 8 cores; 6 planes/core):
  log_img = ln(x + 1e-8)
  illum   = sum_s w_s * gauss2d_s(log_img)        (sigmas 2, 4, 8)
  refl    = log_img - illum
  out     = clip(exp((refl - mean) / (std_ddof1 + 1e-8)), 0, 1)

The 2-D Gaussian is separable: gauss2d_s(X) = U_s @ X @ U_s with U_s the
banded symmetric Toeplitz matrix of the 1-D kernel. Each pass is computed on
the TensorEngine as  pass(D) = D^T @ V  (lhsT = D blocks, rhs = V blocks), so
two passes give V^T X V = V X V with no explicit transposes. Folding
sqrt(w_s) into V_s makes illum a single PSUM accumulation in pass 2. The
banded structure limits each matmul's moving free dim to 128+2c columns.
Matmul operands are fp16 (full-rate PE, ~5e-4 rel precision); everything
else stays fp32.
"""

import numpy as np

N_CORES = 8
NPLANES = 6          # 2 batch images x 3 channels per core
H = W = 512
P = 128
NB = H // P          # 4 row blocks
CS = (6, 12, 24)     # band half-widths for sigma 2, 4, 8 (K = 13, 25, 49)
EPS = 1e-8
NPIX = H * W

_PROGRAM_CACHE = {}

# engine-balance knobs (tuned on HW):
#   L16_ON_ACT: produce the fp16 log copy on ScalarE instead of VectorE
#   EVAC_DVE_MOD: evacuations with (index % 12) < EVAC_DVE_MOD go to DVE
L16_ON_ACT = False
EVAC_DVE_MOD = 6
SBUF_BUFS = 2


def _ncol(kb, c):
    """Output column range that input row block kb touches through a band-c kernel."""
    return max(0, P * kb - c), min(W, P * (kb + 1) + c)


def build_program(reps=1, ablate=()):
    """Build + compile the per-core Bass program. reps>1 wraps the whole
    computation in a hardware loop (for timing by subtraction).
    ablate: dev-only set of stage names to skip ("pe", "gpsimd", "act",
    "evac", "dve") — output becomes wrong; used to attribute HW time."""
    ablate = set(ablate)
    import concourse.bacc as bacc
    import concourse.tile as tile
    from concourse import mybir, bass_isa

    f32 = mybir.dt.float32
    f16 = mybir.dt.float16
    AF = mybir.ActivationFunctionType

    # The activation-table chooser picks the first set containing each
    # function, which puts Ln in "natural_log" and Exp in "exp_and_others" and
    # reloads tables (~2.7us each) every plane. Narrow the cached table map so
    # only the combined "natural_log_exp_and_others" set provides Ln/Exp; then
    # one load serves the whole kernel. (The combined set genuinely contains
    # both functions; this only steers a valid choice.)
    from concourse.hw_specs import get_activation_tables
    _tabs = get_activation_tables("gen3")
    for _name, _fset in _tabs.items():
        if _name != "natural_log_exp_and_others":
            _fset.discard(AF.Ln)
            _fset.discard(AF.Exp)

    nc = bacc.Bacc("TRN2", target_bir_lowering=False, debug=False,
                   num_devices=N_CORES)
    x = nc.declare_dram_parameter("x", [NPLANES, H, W], f32, isOutput=False)
    vs = [nc.declare_dram_parameter(f"v{s}", [H, W], f16, isOutput=False)
          for s in range(3)]
    y = nc.declare_dram_parameter("y", [NPLANES, H, W], f32, isOutput=True)

    with tile.TileContext(nc) as tc:
        with (
            tc.tile_pool(name="consts", bufs=1) as consts,
            tc.tile_pool(name="xin", bufs=SBUF_BUFS) as xpool,
            tc.tile_pool(name="logp", bufs=SBUF_BUFS) as lpool,
            tc.tile_pool(name="log16", bufs=SBUF_BUFS) as l16pool,
            tc.tile_pool(name="apool", bufs=SBUF_BUFS) as apool,
            tc.tile_pool(name="refl", bufs=SBUF_BUFS) as rpool,
            tc.tile_pool(name="yout", bufs=SBUF_BUFS) as ypool,
            tc.tile_pool(name="small", bufs=2) as spool,
            tc.tile_pool(name="psA", bufs=5, space="PSUM") as psA,
            tc.tile_pool(name="psI", bufs=2, space="PSUM") as psIp,
            tc.tile_pool(name="psS", bufs=1, space="PSUM") as psSp,
        ):
            # Banded blur matrices, resident for the whole kernel.
            # Layout [p, kb, n]: matrix row = kb*128 + p.
            V16 = []
            for s in range(3):
                vt = consts.tile([P, NB, W], f16, tag=f"v{s}")
                nc.sync.dma_start(
                    out=vt, in_=vs[s].rearrange("(kb p) n -> p kb n", p=P))
                V16.append(vt)
            epst = consts.tile([P, 1], f32, tag="eps")
            nc.vector.memset(epst, EPS)
            ones16 = consts.tile([P, P], f16, tag="ones16")
            nc.vector.memset(ones16, 1.0 / P)

            def emit_planes():
                state = {}

                def front(p):
                    # load -> ln -> fp16 copy -> pass 1 (A_s = L^T V_s)
                    xt = xpool.tile([P, NB, W], f32, tag="x")
                    nc.sync.dma_start(
                        out=xt, in_=x[p].rearrange("(kb q) w -> q kb w", q=P))
                    lt = lpool.tile([P, NB, W], f32, tag="l")
                    if "act" in ablate:
                        nc.scalar.copy(out=lt, in_=xt)
                    else:
                        nc.scalar.activation(out=lt, in_=xt, func=AF.Ln,
                                             bias=epst, scale=1.0)
                    l16 = l16pool.tile([P, NB, W], f16, tag="l16")
                    if L16_ON_ACT:
                        nc.scalar.copy(out=l16, in_=lt)
                    else:
                        nc.vector.tensor_copy(out=l16, in_=lt)

                    A16 = [apool.tile([P, NB, W], f16, tag=f"a{s}", name=f"a16_{s}")
                           for s in range(3)]
                    if "evac" in ablate:
                        for s in range(3):
                            nc.gpsimd.memset(A16[s], 0.5)
                    nevac = 0
                    if "pe" in ablate:
                        dummy = lpool.tile([P, W], f32, tag="dummy", name="dummy")
                        nc.vector.memset(dummy, 0.25)
                    for mb in range(NB):
                        if "pe" in ablate:
                            ps = [dummy, dummy, dummy]
                        else:
                            ps = [psA.tile([P, W], f32, tag="ps", name=f"ps{i}")
                                  for i in range(3)]
                        if "pe" not in ablate:
                            for s in range(3):
                                for kb in range(NB):
                                    lo, hi = _ncol(kb, CS[s])
                                    nc.tensor.matmul(
                                        ps[s][:, lo:hi],
                                        l16[:, kb, P * mb:P * (mb + 1)],
                                        V16[s][:, kb, lo:hi],
                                        start=(kb == 0), stop=(kb == NB - 1),
                                    )
                        for s in range(3):
                            # evacuate PSUM -> SBUF fp16; 6 on DVE, 6 on ACT
                            if "evac" in ablate:
                                continue
                            if nevac % 12 >= EVAC_DVE_MOD:
                                nc.scalar.copy(out=A16[s][:, mb, :], in_=ps[s])
                            else:
                                nc.vector.tensor_copy(out=A16[s][:, mb, :],
                                                      in_=ps[s])
                            nevac += 1
                    state[p] = (lt, A16)

                def back(p):
                    # pass 2 (illum = sum_s A_s^T V_s) -> refl -> stats -> out
                    lt, A16 = state.pop(p)
                    rt = rpool.tile([P, NB, W], f32, tag="r")
                    st6 = spool.tile([P, NB, 6], f32, tag="st6")
                    for mb in range(NB):
                        if "pe" in ablate:
                            psi = lt[:, mb, :]
                        else:
                            psi = psIp.tile([P, W], f32, tag="psi")
                        if "pe" not in ablate:
                            first = True
                            for s in range(3):
                                for kb in range(NB):
                                    lo, hi = _ncol(kb, CS[s])
                                    nc.tensor.matmul(
                                        psi[:, lo:hi],
                                        A16[s][:, kb, P * mb:P * (mb + 1)],
                                        V16[s][:, kb, lo:hi],
                                        start=first,
                                        stop=(s == 2 and kb == NB - 1),
                                    )
                                    first = False
                        nc.vector.tensor_sub(out=rt[:, mb, :],
                                             in0=lt[:, mb, :], in1=psi)
                        nc.vector.bn_stats(out=st6[:, mb, :], in_=rt[:, mb, :])

                    # plane-wide mean/var: per-partition bn stats, then a
                    # ones-weights matmul sums [mean_p, E[x^2]_p] across
                    # partitions AND broadcasts the result to all partitions
                    # (out[m, n] = sum_k (1/P) * t2[k, n] for every m).
                    mv = spool.tile([P, 2], f32, tag="mv")
                    nc.vector.bn_aggr(out=mv, in_=st6)
                    t2 = spool.tile([P, 2], f16, tag="t2")
                    nc.vector.tensor_mul(out=t2[:, 1:2], in0=mv[:, 0:1],
                                         in1=mv[:, 0:1])
                    nc.vector.tensor_add(out=t2[:, 1:2], in0=t2[:, 1:2],
                                         in1=mv[:, 1:2])
                    nc.vector.tensor_copy(out=t2[:, 0:1], in_=mv[:, 0:1])
                    psS = psSp.tile([P, 2], f32, tag="psS")
                    nc.tensor.matmul(psS, ones16, t2, start=True, stop=True)

                    fin = spool.tile([P, 4], f32, tag="fin")
                    mean = fin[:, 0:1]
                    tmp = fin[:, 1:2]   # var -> std -> std+eps
                    rs = fin[:, 2:3]
                    nbv = fin[:, 3:4]
                    nc.vector.tensor_copy(out=mean, in_=psS[:, 0:1])
                    sq = spool.tile([P, 1], f32, tag="sq")
                    nc.vector.tensor_mul(out=sq, in0=mean, in1=mean)
                    nc.vector.tensor_sub(out=tmp, in0=psS[:, 1:2], in1=sq)
                    # std = exp(0.5*ln(var * N/(N-1)))  (ddof=1), avoiding the
                    # sqrt table set; Ln/Exp share one ACT table set.
                    nc.scalar.activation(out=tmp, in_=tmp, func=AF.Ln,
                                         scale=float(NPIX) / (NPIX - 1))
                    nc.scalar.activation(out=tmp, in_=tmp, func=AF.Exp,
                                         scale=0.5)
                    nc.vector.tensor_scalar_add(out=tmp, in0=tmp, scalar1=EPS)
                    nc.vector.reciprocal(out=rs, in_=tmp)
                    nc.vector.tensor_mul(out=nbv, in0=mean, in1=rs)
                    nc.vector.tensor_scalar_mul(out=nbv, in0=nbv, scalar1=-1.0)

                    yt = ypool.tile([P, NB, W], f32, tag="y")
                    if "act" in ablate:
                        nc.scalar.copy(out=yt, in_=rt)
                    else:
                        nc.scalar.activation(out=yt, in_=rt, func=AF.Exp,
                                             bias=nbv, scale=rs)
                    nc.vector.tensor_scalar_min(out=yt, in0=yt, scalar1=1.0)
                    nc.sync.dma_start(
                        out=y[p].rearrange("(kb q) w -> q kb w", q=P), in_=yt)

                # software-pipelined: pass 1 of plane p overlaps pass 2 of p-1
                for p in range(NPLANES + 1):
                    if p < NPLANES:
                        front(p)
                    if p >= 1:
                        back(p - 1)

            if isinstance(reps, str) and reps.startswith("u"):
                for _ in range(int(reps[1:])):
                    emit_planes()
            elif reps == 1:
                emit_planes()
            else:
                from concourse import mybir as _mb
                with tc.For_i(0, reps, 1,
                              hint_engines=(_mb.EngineType.PE,)):
                    emit_planes()

    nc.compile()
    return nc


def get_program(reps=1):
    if reps not in _PROGRAM_CACHE:
        _PROGRAM_CACHE[reps] = build_program(reps)
    return _PROGRAM_CACHE[reps]


def build_v_matrices(k0, k1, k2):
    """fp16 banded Toeplitz matrices sqrt(w_s) * toeplitz(u_s) from the
    reference's 2-D depthwise kernels (u_s = column sums of the normalized
    2-D kernel, exact by separability)."""
    w = np.array([1.0, 0.75, 0.5], dtype=np.float64)
    w /= w.sum()
    out = []
    for s, k2d in enumerate((k0, k1, k2)):
        g = np.asarray(k2d)[0, 0].astype(np.float64)
        u = g.sum(axis=0)
        c = len(u) // 2
        V = np.zeros((H, W), dtype=np.float64)
        for d in range(-c, c + 1):
            V += np.diag(np.full(H - abs(d), u[c + d]), k=d)
        V *= np.sqrt(w[s])
        out.append(V.astype(np.float16))
    return out


def kernel(rgb_image, k0, k1, k2):
    from concourse.bass_utils import run_bass_kernel_spmd

    nc = get_program()
    v16 = build_v_matrices(k0, k1, k2)
    xs = np.ascontiguousarray(np.asarray(rgb_image, dtype=np.float32))
    B = xs.shape[0]
    per_core = B // N_CORES
    in_maps = []
    for c in range(N_CORES):
        m = {"x": xs[c * per_core:(c + 1) * per_core].reshape(NPLANES, H, W)}
        for s in range(3):
            m[f"v{s}"] = v16[s]
        in_maps.append(m)
    res = run_bass_kernel_spmd(nc, in_maps, list(range(N_CORES)))
    out = np.empty((B, 3, H, W), dtype=np.float32)
    for c in range(N_CORES):
        out[c * per_core:(c + 1) * per_core] = (
            res.results[c]["y"].reshape(per_core, 3, H, W))
    return out
